# revision 26
# baseline (speedup 1.0000x reference)
"""BDCovpool + Triuvec kernel for Trainium2 (8 NeuronCores, data-parallel).

Math (per sample b, x[b]: [M=196, D=512], t: scalar):
  gram[i,j] = sum_m x[m,i] x[m,j]           (D x D)
  d[i]      = gram[i,i]
  dpre      = d[i] + d[j] - 2 gram
  dcov      = sqrt(exp(t) * relu(dpre) + 1e-5)
  cent      = dcov - rowmean - colmean + totmean   (dcov symmetric -> row==col)
  out       = upper triangle of cent, row-major (131328 per sample)

Device strategy per core (32 samples), bf16/f16 datapath:
  x is shipped bf16.  The k-dim (m=196) splits 128 + 70 where the 70-row
  chunk carries two extra host-built rows: lhs [xb; u; -1], rhs
  [xb; -1; u], u = (d-196)/2 in bf16 (d precomputed from the bf16 x on
  the host).  Those rows contribute -u_i - u_j to the gram, so
      PSUM pg = gram - u_i - u_j - gamma*I_blockdiag
  and dcov = Sqrt(pg * (-2 e^t) + (392 e^t + eps)) with a constant bias;
  the gamma shift keeps the (cancelled, ~0) diagonal argument strictly
  positive.  Only the upper-triangle row blocks [128 x (512-128r)] are
  computed; each Sqrt writes f16 straight into the packed output tile.
  Row sums (for the centering statistics) combine three pieces:
   - diag-block column sums     (PE matmuls vs a ones column; the block
     is symmetric so colsum == rowsum)
   - earlier-block column sums  (PE, the mirrored lower-triangle part)
   - off-diag tail row sums     (DVE tensor_scalar accumulate, 4x mode)
  merged into 8 extra f16 (bitcast f32) columns of the output tile.
  4 consecutive samples share each input/output DMA (strided dram APs)
  to amortize the serialized HWDGE descriptor-generation cost.
Host: shard B across 8 cores, cast x to bf16 + build u rows, then during
un-sharding apply the (affine) double-centering from the shipped row
sums: cent = dcov - rm_i - rm_j + K, fix the gamma-shifted diagonal, and
gather the row-major upper triangle.
"""

import numpy as np

B, M, D = 256, 196, 512
NCORES = 8
S = B // NCORES  # samples per core
P = 128
NCH = D // P  # 4 row chunks
MB = M - P  # 68 rows in second k-chunk
KP = MB + 2  # 70 rows incl. u and -1
GAMMA = 256.0  # diagonal shift (bf16-exact)
EPS = 1e-5
KC = float(M)  # E[d] = M
CH_W = [D - P * r for r in range(NCH)]  # 512, 384, 256, 128
# packed block offsets; order 0,1,3,2 keeps each [128 x w] f32 PSUM block
# inside 2KB bank boundaries
OO = [0, 512, 1024, 896]
OW = 1280
OWX = OW + 8  # + 8 f16 cols holding the f32 [128,4] row sums (bitcast)


def build_nc(n_samples=S, fixup=True):
    import concourse.bass as bass
    import concourse.mybir as mybir
    import concourse.tile as tile

    f32 = mybir.dt.float32
    bf16 = mybir.dt.bfloat16
    f16 = mybir.dt.float16
    u8 = mybir.dt.uint8
    AF = mybir.ActivationFunctionType
    ALU = mybir.AluOpType

    nc = bass.Bass(
        "TRN2", target_bir_lowering=False, debug=False, enable_asserts=False
    )

    xa_d = nc.dram_tensor("xa", [n_samples, P, D], bf16, kind="ExternalInput").ap()
    xp_d = nc.dram_tensor("xp", [n_samples, KP, 2 * D], bf16, kind="ExternalInput").ap()
    # packed constants: idc = [I | -gamma*I] bf16 (512B), ones f16 (2B),
    # t f32 (4B at byte 516)
    cst_d = nc.dram_tensor("cst", [P, 520], u8, kind="ExternalInput").ap()
    out_d = nc.dram_tensor("out", [n_samples, P, OWX], f16, kind="ExternalOutput").ap()

    with tile.TileContext(nc) as tc:
        with (
            tc.tile_pool(name="const", bufs=1) as cpool,
            tc.tile_pool(name="xa", bufs=6) as xa_pool,
            tc.tile_pool(name="xp", bufs=6) as xp_pool,
            tc.tile_pool(name="sm", bufs=4) as sm_pool,
            tc.tile_pool(name="uo", bufs=4) as uo_pool,
            tc.tile_pool(name="pg", bufs=4, space="PSUM") as pg_pool,
            tc.tile_pool(name="psm", bufs=4, space="PSUM") as psm_pool,
        ):
            cst = cpool.tile([P, 520], u8, tag="cst")
            nc.sync.dma_start(cst[:], cst_d[:])
            idc = cst[:, 0:512].bitcast(bf16)
            cbf = cst[:, 512:514].bitcast(f16)
            t128 = cst[:, 516:520].bitcast(f32)
            et128 = cpool.tile([P, 1], f32, tag="et128")
            nc.scalar.activation(et128[:], t128[:], AF.Exp)
            scb = cpool.tile([P, 2], f32, tag="scb")
            # scb col0 = -2 e^t (sqrt scale); col1 = 392 e^t + 1e-5 (bias)
            nc.vector.tensor_scalar_mul(scb[:, 0:1], et128[:], -2.0)
            nc.vector.tensor_scalar(
                scb[:, 1:2], et128[:], 2.0 * KC, EPS, ALU.mult, ALU.add
            )
            sc_ap = scb[:, 0:1]
            bias_ap = scb[:, 1:2]

            def load_group(s, ns):
                """One DMA per dram tensor for `ns` consecutive samples."""
                xa = xa_pool.tile([P, ns * D], bf16, tag="xa")
                nc.sync.dma_start(
                    xa[:].rearrange("p (s c) -> p s c", s=ns),
                    xa_d[s : s + ns].rearrange("s p c -> p s c"),
                )
                xp = xp_pool.tile([KP, ns * 2 * D], bf16, tag="xp")
                nc.sync.dma_start(
                    xp[:].rearrange("p (s c) -> p s c", s=ns),
                    xp_d[s : s + ns].rearrange("s p c -> p s c"),
                )
                ob = uo_pool.tile([P, ns * OWX], f16, tag="ob")
                return xa, xp, ob

            def stage1(k, xa2, xp2, ob2):
                xa = xa2[:, k * D : (k + 1) * D]
                xp = xp2[:, k * 2 * D : (k + 1) * 2 * D]
                ob = ob2[:, k * OWX : (k + 1) * OWX]
                acc4 = sm_pool.tile([P, 4], f32, tag="acc4")
                smt = psm_pool.tile([P, 4], f32, tag="smt")
                sm = smt[:, 0:4]
                dcs = [ob[:, OO[r] : OO[r] + CH_W[r]] for r in range(NCH)]
                for r in range(NCH):
                    w = CH_W[r]
                    sl = slice(P * r, P * (r + 1))
                    pgt = pg_pool.tile([P, D], f32, tag="pg")
                    pgr = pgt[:, 0:w]
                    nc.tensor.matmul(
                        pgr, xa[:, sl], xa[:, P * r : D],
                        start=True, stop=False,
                    )
                    nc.tensor.matmul(
                        pgr[:, 0:P], idc[:, 0:P], idc[:, P : 2 * P],
                        start=False, stop=False, skip_group_check=True,
                    )
                    nc.tensor.matmul(
                        pgr, xp[:, sl], xp[:, D + P * r : 2 * D],
                        start=False, stop=True,
                    )
                    dc = dcs[r]
                    nc.scalar.activation(
                        dc, pgr, AF.Sqrt, bias=bias_ap, scale=sc_ap
                    )
                    # row sums col r: diag-block colsum (PE) + earlier-block
                    # colsums (PE) + off-diag tail accum (DVE, below)
                    nc.tensor.matmul(
                        sm[:, r : r + 1], dc[:, 0:P], cbf[:],
                        start=True, stop=(r == 0),
                    )
                    if r < NCH - 1:
                        nc.vector.tensor_scalar(
                            dc[:, P:w], dc[:, P:w], 1.0, None, ALU.mult, ALU.add,
                            accum_out=acc4[:, r : r + 1],
                        )
                    for rp in range(r):
                        loc = P * (r - rp)
                        nc.tensor.matmul(
                            sm[:, r : r + 1],
                            dcs[rp][:, loc : loc + P],
                            cbf[:],
                            start=False, stop=(rp == r - 1),
                        )
                # merge rowsum pieces into the output's trailing f32 cols
                rsv = ob[:, OW : OW + 8].bitcast(f32)
                nc.vector.tensor_tensor(
                    rsv[:, 0:3], acc4[:, 0:3], sm[:, 0:3], ALU.add
                )
                nc.vector.tensor_copy(rsv[:, 3:4], sm[:, 3:4])

            def store(prev):
                ps, pns, pob = prev
                nc.sync.dma_start(
                    out_d[ps : ps + pns].rearrange("s p c -> p s c"),
                    pob[:].rearrange("p (s c) -> p s c", s=pns),
                )

            GRP = 4
            prev = None
            for s in range(0, n_samples, GRP):
                ns = min(GRP, n_samples - s)
                xa2, xp2, ob2 = load_group(s, ns)
                for k in range(ns):
                    stage1(k, xa2, xp2, ob2)
                if prev is not None:
                    store(prev)
                prev = (s, ns, ob2)
            store(prev)

    # This walrus build accepts at most ONE sync wait per instruction.
    # Tile may attach several; hoist each extra wait onto its own no-op
    # placed just before the instruction (same engine, so ordering holds).
    if fixup:
        import concourse.mybir as mybir
        import bass_rust as _br

        for f in nc.m.functions:
            for blk in f.blocks:
                out_list = []
                changed = False
                for ins in blk.instructions:
                    si = getattr(ins, "sync_info", None)
                    if (
                        type(ins).__name__ != "InstNoOp"
                        and si is not None
                        and si.on_wait
                        and len(si.on_wait) > 1
                        and getattr(ins, "engine", None) is not None
                    ):
                        for j, w in enumerate(si.on_wait[:-1]):
                            nop = _br.InstNoOp(
                                name=f"I-w{j}-{ins.name}",
                                engine=ins.engine,
                                ins=[],
                                outs=[],
                            )
                            nop.sync_info = mybir.SyncInfo(on_wait=[w], on_update=[])
                            out_list.append(nop)
                        ins.sync_info = mybir.SyncInfo(
                            on_wait=[si.on_wait[-1]], on_update=list(si.on_update)
                        )
                        changed = True
                    out_list.append(ins)
                if changed:
                    blk.instructions = out_list
    return nc


def make_cst(t):
    """Packed per-run constants + the diagonal offset delta."""
    import ml_dtypes

    et = np.float32(np.exp(np.float32(np.asarray(t).reshape(-1)[0])))
    cval = np.float32(2.0 * GAMMA * et + EPS)
    delta = np.float32(np.sqrt(cval) - np.sqrt(np.float32(EPS)))
    idc = np.zeros((P, 2 * P), dtype=np.float32)
    idc[:, 0:P] = np.eye(P)
    idc[:, P : 2 * P] = -GAMMA * np.eye(P)
    cst = np.zeros((P, 520), dtype=np.uint8)
    cst[:, 0:512] = idc.astype(ml_dtypes.bfloat16).view(np.uint8)
    cst[:, 512:514] = np.ones((P, 1), dtype=np.float16).view(np.uint8)
    t128 = np.broadcast_to(np.asarray(t, dtype=np.float32).reshape(1, 1), (P, 1))
    cst[:, 516:520] = np.ascontiguousarray(t128).view(np.uint8)
    return cst, float(delta)


def prep_x(x):
    """Full x [B, M, D] f32 -> (xa bf16 [B,P,D], xp bf16 [B,KP,2D])."""
    import ml_dtypes

    xb16 = x.astype(ml_dtypes.bfloat16)
    xa = np.ascontiguousarray(xb16[:, 0:P, :])
    xbt = xb16[:, P:M, :]  # [B, 68, D]
    # u = (d - 196)/2 from the bf16-rounded x (consistent with device gram)
    d = np.square(xb16.astype(np.float32)).sum(axis=1)  # [B, D] f32
    u = ((d - KC) * 0.5).astype(ml_dtypes.bfloat16)
    xp = np.empty((x.shape[0], KP, 2 * D), dtype=ml_dtypes.bfloat16)
    xp[:, 0:MB, 0:D] = xbt
    xp[:, MB, 0:D] = u
    xp[:, MB + 1, 0:D] = ml_dtypes.bfloat16(-1.0)
    xp[:, 0:MB, D : 2 * D] = xbt
    xp[:, MB, D : 2 * D] = ml_dtypes.bfloat16(-1.0)
    xp[:, MB + 1, D : 2 * D] = u
    return xa, xp


def make_in_maps(x, t):
    cst, delta = make_cst(t)
    xa, xp = prep_x(np.ascontiguousarray(x, dtype=np.float32))
    in_maps = []
    for c in range(NCORES):
        sl = slice(c * S, (c + 1) * S)
        in_maps.append({"xa": xa[sl], "xp": xp[sl], "cst": cst})
    return in_maps, delta


# triu assembly indices (static)
_TRIU_ROWSTART = np.zeros(D + 1, dtype=np.int64)
for _i in range(D):
    _TRIU_ROWSTART[_i + 1] = _TRIU_ROWSTART[_i] + (D - _i)
TRIU_LEN = int(_TRIU_ROWSTART[D])  # 131328


def assemble(dev_out, delta):
    """[n, P, OWX] f16 device blocks+rowsums -> centered [n, 131328] triu."""
    n = dev_out.shape[0]
    dev_out = np.asarray(dev_out)
    rs = dev_out[:, :, OW : OW + 8].copy().view(np.float32)  # [n, P, 4]
    dc = dev_out[:, :, 0:OW].astype(np.float32)
    # rs[:, p, c] belongs to row index i = 128c + p
    c_full = rs.transpose(0, 2, 1).reshape(n, D) * (1.0 / D)  # rowmeans
    mu_true = (rs.sum(axis=(1, 2)) - D * delta) / (D * D)
    K = (mu_true + 2.0 * delta / D)[:, None]  # cent = dc - rm_i - rm_j + K
    out = np.empty((n, TRIU_LEN), dtype=np.float32)
    for r in range(NCH):
        for p in range(P):
            i = P * r + p
            st = _TRIU_ROWSTART[i]
            ln = D - i
            row = dc[:, p, OO[r] + p : OO[r] + p + ln]
            out[:, st : st + ln] = row - c_full[:, i : i + 1] - c_full[:, i:] + K
            out[:, st] -= delta  # gamma-shifted diagonal entry
    return out


_CACHE = {}


def kernel(**inputs):
    import concourse.bass_utils as bass_utils

    x = np.ascontiguousarray(inputs["x"], dtype=np.float32)
    t = np.asarray(inputs["t"], dtype=np.float32)
    assert x.shape == (B, M, D)

    if "nc" not in _CACHE:
        _CACHE["nc"] = build_nc(S)
    nc = _CACHE["nc"]
    in_maps, delta = make_in_maps(x, t)
    res = bass_utils.run_bass_kernel_spmd(nc, in_maps, core_ids=list(range(NCORES)))
    full = np.empty((B, TRIU_LEN), dtype=np.float32)
    for c in range(NCORES):
        full[c * S : (c + 1) * S] = assemble(res.results[c]["out"], delta)
    return full


# revision 30
# speedup vs baseline: 1.0613x; 1.0613x over previous
"""BDCovpool + Triuvec kernel for Trainium2 (8 NeuronCores, data-parallel).

Math (per sample b, x[b]: [M=196, D=512], t: scalar):
  gram[i,j] = sum_m x[m,i] x[m,j]           (D x D)
  d[i]      = gram[i,i]
  dpre      = d[i] + d[j] - 2 gram
  dcov      = sqrt(exp(t) * relu(dpre) + 1e-5)
  cent      = dcov - rowmean - colmean + totmean   (dcov symmetric -> row==col)
  out       = upper triangle of cent, row-major (131328 per sample)

Device strategy per core (32 samples), bf16/f16 datapath:
  x is shipped bf16.  The k-dim (m=196) splits 128 + 70 where the 70-row
  chunk carries two extra host-built rows: lhs [xb; u; -1], rhs
  [xb; -1; u], u = (d-196)/2 in bf16 (d precomputed from the bf16 x on
  the host).  Those rows contribute -u_i - u_j to the gram, so
      PSUM pg = gram - u_i - u_j - gamma*I_blockdiag
  and dcov = Sqrt(pg * (-2 e^t) + (392 e^t + eps)) with a constant bias;
  the gamma shift keeps the (cancelled, ~0) diagonal argument strictly
  positive.  Only the upper-triangle row blocks [128 x (512-128r)] are
  computed; each Sqrt writes f16 straight into the packed output tile.
  Row sums (for the centering statistics) combine three pieces:
   - diag-block column sums     (PE matmuls vs a ones column; the block
     is symmetric so colsum == rowsum)
   - earlier-block column sums  (PE, the mirrored lower-triangle part)
   - off-diag tail row sums     (DVE tensor_scalar accumulate, 4x mode)
  merged into 8 extra f16 (bitcast f32) columns of the output tile.
  4 consecutive samples share each input/output DMA (strided dram APs)
  to amortize the serialized HWDGE descriptor-generation cost.
Host: shard B across 8 cores, cast x to bf16 + build u rows, then during
un-sharding apply the (affine) double-centering from the shipped row
sums: cent = dcov - rm_i - rm_j + K, fix the gamma-shifted diagonal, and
gather the row-major upper triangle.
"""

import numpy as np

B, M, D = 256, 196, 512
NCORES = 8
S = B // NCORES  # samples per core
P = 128
NCH = D // P  # 4 row chunks
MB = M - P  # 68 rows in second k-chunk
KP = MB + 2  # 70 rows incl. u and -1
GAMMA = 256.0  # diagonal shift (bf16-exact)
EPS = 1e-5
KC = float(M)  # E[d] = M
CH_W = [D - P * r for r in range(NCH)]  # 512, 384, 256, 128
# packed block offsets; order 0,1,3,2 keeps each [128 x w] f32 PSUM block
# inside 2KB bank boundaries
OO = [0, 512, 1024, 896]
OW = 1280
OWX = OW + 8  # + 8 f16 cols holding the f32 [128,4] row sums (bitcast)


def build_nc(n_samples=S, fixup=True):
    import concourse.bass as bass
    import concourse.mybir as mybir
    import concourse.tile as tile

    f32 = mybir.dt.float32
    bf16 = mybir.dt.bfloat16
    f16 = mybir.dt.float16
    u8 = mybir.dt.uint8
    AF = mybir.ActivationFunctionType
    ALU = mybir.AluOpType

    nc = bass.Bass(
        "TRN2", target_bir_lowering=False, debug=False, enable_asserts=False
    )

    xa_d = nc.dram_tensor("xa", [n_samples, P, D], bf16, kind="ExternalInput").ap()
    xp_d = nc.dram_tensor("xp", [n_samples, KP, 2 * D], bf16, kind="ExternalInput").ap()
    # packed constants: idc = [I | -gamma*I] bf16 (512B), ones f16 (2B),
    # t f32 (4B at byte 516)
    cst_d = nc.dram_tensor("cst", [P, 520], u8, kind="ExternalInput").ap()
    out_d = nc.dram_tensor("out", [n_samples, P, OWX], f16, kind="ExternalOutput").ap()

    with tile.TileContext(nc) as tc:
        with (
            tc.tile_pool(name="const", bufs=1) as cpool,
            tc.tile_pool(name="xa", bufs=6) as xa_pool,
            tc.tile_pool(name="xp", bufs=6) as xp_pool,
            tc.tile_pool(name="sm", bufs=4) as sm_pool,
            tc.tile_pool(name="uo", bufs=4) as uo_pool,
            tc.tile_pool(name="pg", bufs=4, space="PSUM") as pg_pool,
            tc.tile_pool(name="psm", bufs=4, space="PSUM") as psm_pool,
        ):
            cst = cpool.tile([P, 520], u8, tag="cst")
            nc.sync.dma_start(cst[:], cst_d[:])
            idc = cst[:, 0:512].bitcast(bf16)
            cbf = cst[:, 512:514].bitcast(f16)
            t128 = cst[:, 516:520].bitcast(f32)
            et128 = cpool.tile([P, 1], f32, tag="et128")
            nc.scalar.activation(et128[:], t128[:], AF.Exp)
            scb = cpool.tile([P, 2], f32, tag="scb")
            # scb col0 = -2 e^t (sqrt scale); col1 = 392 e^t + 1e-5 (bias)
            nc.vector.tensor_scalar_mul(scb[:, 0:1], et128[:], -2.0)
            nc.vector.tensor_scalar(
                scb[:, 1:2], et128[:], 2.0 * KC, EPS, ALU.mult, ALU.add
            )
            sc_ap = scb[:, 0:1]
            bias_ap = scb[:, 1:2]

            def load_group(s, ns):
                """One DMA per dram tensor for `ns` consecutive samples."""
                xa = xa_pool.tile([P, ns * D], bf16, tag="xa")
                nc.sync.dma_start(
                    xa[:].rearrange("p (s c) -> p s c", s=ns),
                    xa_d[s : s + ns].rearrange("s p c -> p s c"),
                )
                xp = xp_pool.tile([KP, ns * 2 * D], bf16, tag="xp")
                nc.sync.dma_start(
                    xp[:].rearrange("p (s c) -> p s c", s=ns),
                    xp_d[s : s + ns].rearrange("s p c -> p s c"),
                )
                ob = uo_pool.tile([P, ns * OWX], f16, tag="ob")
                return xa, xp, ob

            def stage1(k, xa2, xp2, ob2):
                xa = xa2[:, k * D : (k + 1) * D]
                xp = xp2[:, k * 2 * D : (k + 1) * 2 * D]
                ob = ob2[:, k * OWX : (k + 1) * OWX]
                acc4 = sm_pool.tile([P, 4], f32, tag="acc4")
                smt = psm_pool.tile([P, 4], f32, tag="smt")
                sm = smt[:, 0:4]
                dcs = [ob[:, OO[r] : OO[r] + CH_W[r]] for r in range(NCH)]
                for r in range(NCH):
                    w = CH_W[r]
                    sl = slice(P * r, P * (r + 1))
                    pgt = pg_pool.tile([P, D], f32, tag="pg")
                    pgr = pgt[:, 0:w]
                    nc.tensor.matmul(
                        pgr, xa[:, sl], xa[:, P * r : D],
                        start=True, stop=False,
                    )
                    nc.tensor.matmul(
                        pgr[:, 0:P], idc[:, 0:P], idc[:, P : 2 * P],
                        start=False, stop=False, skip_group_check=True,
                    )
                    nc.tensor.matmul(
                        pgr, xp[:, sl], xp[:, D + P * r : 2 * D],
                        start=False, stop=True,
                    )
                    dc = dcs[r]
                    nc.scalar.activation(
                        dc, pgr, AF.Sqrt, bias=bias_ap, scale=sc_ap
                    )
                    # row sums col r: diag-block colsum (PE) + earlier-block
                    # colsums (PE) + off-diag tail accum (DVE, below)
                    nc.tensor.matmul(
                        sm[:, r : r + 1], dc[:, 0:P], cbf[:],
                        start=True, stop=(r == 0),
                    )
                    if r < NCH - 1:
                        nc.vector.tensor_scalar(
                            dc[:, P:w], dc[:, P:w], 1.0, None, ALU.mult, ALU.add,
                            accum_out=acc4[:, r : r + 1],
                        )
                    for rp in range(r):
                        loc = P * (r - rp)
                        nc.tensor.matmul(
                            sm[:, r : r + 1],
                            dcs[rp][:, loc : loc + P],
                            cbf[:],
                            start=False, stop=(rp == r - 1),
                        )
                # merge rowsum pieces into the output's trailing f32 cols
                rsv = ob[:, OW : OW + 8].bitcast(f32)
                nc.vector.tensor_tensor(
                    rsv[:, 0:3], acc4[:, 0:3], sm[:, 0:3], ALU.add
                )
                nc.vector.tensor_copy(rsv[:, 3:4], sm[:, 3:4])

            def store(prev, split=False):
                ps, pns, pob = prev
                if not split:
                    nc.sync.dma_start(
                        out_d[ps : ps + pns].rearrange("s p c -> p s c"),
                        pob[:].rearrange("p (s c) -> p s c", s=pns),
                    )
                    return
                # tail store: blocks 0,1 (ready right after their sqrts)
                # first, so only the short remainder trails the last compute
                nc.sync.dma_start(
                    out_d[ps : ps + pns, :, 0:896].rearrange("s p c -> p s c"),
                    pob[:].rearrange("p (s c) -> p s c", c=OWX)[:, :, 0:896],
                )
                nc.sync.dma_start(
                    out_d[ps : ps + pns, :, 896:OWX].rearrange("s p c -> p s c"),
                    pob[:].rearrange("p (s c) -> p s c", c=OWX)[:, :, 896:OWX],
                )

            # group sizes: small first/last groups shorten the pipeline
            # ramp (first sqrt waits on the first loads) and the drain
            # (the final stores); 4-sample groups amortize HWDGE otherwise.
            if n_samples < 12:
                sizes = [1] * n_samples
            else:
                body = n_samples - 8
                sizes = (
                    [1, 1, 2]
                    + [4] * (body // 4)
                    + ([body % 4] if body % 4 else [])
                    + [2, 1, 1]
                )

            prev = None
            s = 0
            for ns in sizes:
                xa2, xp2, ob2 = load_group(s, ns)
                for k in range(ns):
                    stage1(k, xa2, xp2, ob2)
                if prev is not None:
                    store(prev)
                prev = (s, ns, ob2)
                s += ns
            store(prev, split=True)
            assert s == n_samples, (sizes, s)

    # This walrus build accepts at most ONE sync wait per instruction.
    # Tile may attach several; hoist each extra wait onto its own no-op
    # placed just before the instruction (same engine, so ordering holds).
    if fixup:
        import concourse.mybir as mybir
        import bass_rust as _br

        for f in nc.m.functions:
            for blk in f.blocks:
                out_list = []
                changed = False
                for ins in blk.instructions:
                    si = getattr(ins, "sync_info", None)
                    if (
                        type(ins).__name__ != "InstNoOp"
                        and si is not None
                        and si.on_wait
                        and len(si.on_wait) > 1
                        and getattr(ins, "engine", None) is not None
                    ):
                        for j, w in enumerate(si.on_wait[:-1]):
                            nop = _br.InstNoOp(
                                name=f"I-w{j}-{ins.name}",
                                engine=ins.engine,
                                ins=[],
                                outs=[],
                            )
                            nop.sync_info = mybir.SyncInfo(on_wait=[w], on_update=[])
                            out_list.append(nop)
                        ins.sync_info = mybir.SyncInfo(
                            on_wait=[si.on_wait[-1]], on_update=list(si.on_update)
                        )
                        changed = True
                    out_list.append(ins)
                if changed:
                    blk.instructions = out_list
    return nc


def make_cst(t):
    """Packed per-run constants + the diagonal offset delta."""
    import ml_dtypes

    et = np.float32(np.exp(np.float32(np.asarray(t).reshape(-1)[0])))
    cval = np.float32(2.0 * GAMMA * et + EPS)
    delta = np.float32(np.sqrt(cval) - np.sqrt(np.float32(EPS)))
    idc = np.zeros((P, 2 * P), dtype=np.float32)
    idc[:, 0:P] = np.eye(P)
    idc[:, P : 2 * P] = -GAMMA * np.eye(P)
    cst = np.zeros((P, 520), dtype=np.uint8)
    cst[:, 0:512] = idc.astype(ml_dtypes.bfloat16).view(np.uint8)
    cst[:, 512:514] = np.ones((P, 1), dtype=np.float16).view(np.uint8)
    t128 = np.broadcast_to(np.asarray(t, dtype=np.float32).reshape(1, 1), (P, 1))
    cst[:, 516:520] = np.ascontiguousarray(t128).view(np.uint8)
    return cst, float(delta)


def prep_x(x):
    """Full x [B, M, D] f32 -> (xa bf16 [B,P,D], xp bf16 [B,KP,2D])."""
    import ml_dtypes

    xb16 = x.astype(ml_dtypes.bfloat16)
    xa = np.ascontiguousarray(xb16[:, 0:P, :])
    xbt = xb16[:, P:M, :]  # [B, 68, D]
    # u = (d - 196)/2 from the bf16-rounded x (consistent with device gram)
    d = np.square(xb16.astype(np.float32)).sum(axis=1)  # [B, D] f32
    u = ((d - KC) * 0.5).astype(ml_dtypes.bfloat16)
    xp = np.empty((x.shape[0], KP, 2 * D), dtype=ml_dtypes.bfloat16)
    xp[:, 0:MB, 0:D] = xbt
    xp[:, MB, 0:D] = u
    xp[:, MB + 1, 0:D] = ml_dtypes.bfloat16(-1.0)
    xp[:, 0:MB, D : 2 * D] = xbt
    xp[:, MB, D : 2 * D] = ml_dtypes.bfloat16(-1.0)
    xp[:, MB + 1, D : 2 * D] = u
    return xa, xp


def make_in_maps(x, t):
    cst, delta = make_cst(t)
    xa, xp = prep_x(np.ascontiguousarray(x, dtype=np.float32))
    in_maps = []
    for c in range(NCORES):
        sl = slice(c * S, (c + 1) * S)
        in_maps.append({"xa": xa[sl], "xp": xp[sl], "cst": cst})
    return in_maps, delta


# triu assembly indices (static)
_TRIU_ROWSTART = np.zeros(D + 1, dtype=np.int64)
for _i in range(D):
    _TRIU_ROWSTART[_i + 1] = _TRIU_ROWSTART[_i] + (D - _i)
TRIU_LEN = int(_TRIU_ROWSTART[D])  # 131328


def assemble(dev_out, delta):
    """[n, P, OWX] f16 device blocks+rowsums -> centered [n, 131328] triu."""
    n = dev_out.shape[0]
    dev_out = np.asarray(dev_out)
    rs = dev_out[:, :, OW : OW + 8].copy().view(np.float32)  # [n, P, 4]
    dc = dev_out[:, :, 0:OW].astype(np.float32)
    # rs[:, p, c] belongs to row index i = 128c + p
    c_full = rs.transpose(0, 2, 1).reshape(n, D) * (1.0 / D)  # rowmeans
    mu_true = (rs.sum(axis=(1, 2)) - D * delta) / (D * D)
    K = (mu_true + 2.0 * delta / D)[:, None]  # cent = dc - rm_i - rm_j + K
    out = np.empty((n, TRIU_LEN), dtype=np.float32)
    for r in range(NCH):
        for p in range(P):
            i = P * r + p
            st = _TRIU_ROWSTART[i]
            ln = D - i
            row = dc[:, p, OO[r] + p : OO[r] + p + ln]
            out[:, st : st + ln] = row - c_full[:, i : i + 1] - c_full[:, i:] + K
            out[:, st] -= delta  # gamma-shifted diagonal entry
    return out


_CACHE = {}


def kernel(**inputs):
    import concourse.bass_utils as bass_utils

    x = np.ascontiguousarray(inputs["x"], dtype=np.float32)
    t = np.asarray(inputs["t"], dtype=np.float32)
    assert x.shape == (B, M, D)

    if "nc" not in _CACHE:
        _CACHE["nc"] = build_nc(S)
    nc = _CACHE["nc"]
    in_maps, delta = make_in_maps(x, t)
    res = bass_utils.run_bass_kernel_spmd(nc, in_maps, core_ids=list(range(NCORES)))
    full = np.empty((B, TRIU_LEN), dtype=np.float32)
    for c in range(NCORES):
        full[c * S : (c + 1) * S] = assemble(res.results[c]["out"], delta)
    return full


# revision 48
# speedup vs baseline: 1.2111x; 1.1411x over previous
"""BDCovpool + Triuvec kernel for Trainium2 (8 NeuronCores, data-parallel).

Math (per sample b, x[b]: [M=196, D=512], t: scalar):
  gram[i,j] = sum_m x[m,i] x[m,j]           (D x D)
  d[i]      = gram[i,i]
  dpre      = d[i] + d[j] - 2 gram
  dcov      = sqrt(exp(t) * relu(dpre) + 1e-5)
  cent      = dcov - rowmean - colmean + totmean   (dcov symmetric -> row==col)
  out       = upper triangle of cent, row-major (131328 per sample)

Device strategy per core (32 samples), bf16/f16 datapath:
  x is shipped bf16.  The k-dim (m=196) splits 128 + 70 where the 70-row
  chunk carries two extra host-built rows: lhs [xb; u; -1], rhs
  [xb; -1; u], u = (d-196)/2 in bf16 (d precomputed from the bf16 x on
  the host).  Those rows contribute -u_i - u_j to the gram, so
      PSUM pg = gram - u_i - u_j - gamma*I_blockdiag
  and dcov = Sqrt(pg * (-2 e^t) + (392 e^t + eps)) with a constant bias;
  the gamma shift keeps the (cancelled, ~0) diagonal argument strictly
  positive.  Only the upper-triangle row blocks [128 x (512-128r)] are
  computed; each Sqrt writes f16 straight into the packed output tile.
  Row sums (for the centering statistics) combine three pieces:
   - diag-block column sums     (PE matmuls vs a ones column; the block
     is symmetric so colsum == rowsum)
   - earlier-block column sums  (PE, the mirrored lower-triangle part)
   - off-diag tail row sums     (DVE tensor_scalar accumulate, 4x mode)
  merged into 8 extra f16 (bitcast f32) columns of the output tile.
  4 consecutive samples share each input/output DMA (strided dram APs)
  to amortize the serialized HWDGE descriptor-generation cost.
Host: shard B across 8 cores, cast x to bf16 + build u rows, then during
un-sharding apply the (affine) double-centering from the shipped row
sums: cent = dcov - rm_i - rm_j + K, fix the gamma-shifted diagonal, and
gather the row-major upper triangle.
"""

import numpy as np

B, M, D = 256, 196, 512
NCORES = 8
S = B // NCORES  # samples per core
P = 128
NCH = D // P  # 4 row chunks
MB = M - P  # 68 rows in second k-chunk
KP = MB + 2  # 70 rows incl. u and -1
GAMMA = 256.0  # diagonal shift (bf16-exact)
EPS = 1e-5
KC = float(M)  # E[d] = M
CH_W = [D - P * r for r in range(NCH)]  # 512, 384, 256, 128
# packed block offsets; order 0,1,3,2 keeps each [128 x w] f32 PSUM block
# inside 2KB bank boundaries
OO = [0, 512, 1024, 896]
OW = 1280
OWX = OW + 8  # + 8 f16 cols holding the f32 [128,4] row sums (bitcast)


def build_nc(n_samples=S, fixup=True):
    import concourse.bass as bass
    import concourse.mybir as mybir
    import concourse.tile as tile

    f32 = mybir.dt.float32
    bf16 = mybir.dt.bfloat16
    f16 = mybir.dt.float16
    u8 = mybir.dt.uint8
    AF = mybir.ActivationFunctionType
    ALU = mybir.AluOpType

    nc = bass.Bass(
        "TRN2", target_bir_lowering=False, debug=False, enable_asserts=False
    )

    xa_d = nc.dram_tensor("xa", [n_samples, P, D], bf16, kind="ExternalInput").ap()
    xp_d = nc.dram_tensor("xp", [n_samples, KP, 2 * D], bf16, kind="ExternalInput").ap()
    # packed constants: idc = [I | -gamma*I] bf16 (512B), ones f16 (2B),
    # t f32 (4B at byte 516)
    cst_d = nc.dram_tensor("cst", [P, 520], u8, kind="ExternalInput").ap()
    out_d = nc.dram_tensor("out", [n_samples, P, OWX], f16, kind="ExternalOutput").ap()

    with tile.TileContext(nc) as tc:
        with (
            tc.tile_pool(name="const", bufs=1) as cpool,
            tc.tile_pool(name="xa", bufs=6) as xa_pool,
            tc.tile_pool(name="xp", bufs=6) as xp_pool,
            tc.tile_pool(name="sm", bufs=4) as sm_pool,
            tc.tile_pool(name="uo", bufs=10) as uo_pool,
            tc.tile_pool(name="pg", bufs=3, space="PSUM") as pg_pool,
            tc.tile_pool(name="pgb", bufs=2, space="PSUM") as pgb_pool,
            tc.tile_pool(name="psm", bufs=3, space="PSUM") as psm_pool,
        ):
            cst = cpool.tile([P, 520], u8, tag="cst")
            idc = cst[:, 0:512].bitcast(bf16)
            cbf = cst[:, 512:514].bitcast(f16)
            t128 = cst[:, 516:520].bitcast(f32)
            et128 = cpool.tile([P, 1], f32, tag="et128")
            scb = cpool.tile([P, 2], f32, tag="scb")
            # scb col0 = -2 e^t (sqrt scale); col1 = 392 e^t + 1e-5 (bias);
            # filled after the first loads are issued (see main loop)
            sc_ap = scb[:, 0:1]
            bias_ap = scb[:, 1:2]

            def load_group(s, ns):
                """One DMA per dram tensor for `ns` consecutive samples."""
                xa = xa_pool.tile([P, ns * D], bf16, tag="xa")
                nc.sync.dma_start(
                    xa[:].rearrange("p (s c) -> p s c", s=ns),
                    xa_d[s : s + ns].rearrange("s p c -> p s c"),
                )
                xp = xp_pool.tile([KP, ns * 2 * D], bf16, tag="xp")
                nc.sync.dma_start(
                    xp[:].rearrange("p (s c) -> p s c", s=ns),
                    xp_d[s : s + ns].rearrange("s p c -> p s c"),
                )
                ob = uo_pool.tile([P, ns * OWX], f16, tag="ob")
                return xa, xp, ob

            def stage1(k, xa2, xp2, ob2):
                xa = xa2[:, k * D : (k + 1) * D]
                xp = xp2[:, k * 2 * D : (k + 1) * 2 * D]
                ob = ob2[:, k * OWX : (k + 1) * OWX]
                acc4 = sm_pool.tile([P, 4], f32, tag="acc4")
                smt = psm_pool.tile([P, 4], f32, tag="smt")
                sm = smt[:, 0:4]
                dcs = [ob[:, OO[r] : OO[r] + CH_W[r]] for r in range(NCH)]

                def gram(r, pgr):
                    sl = slice(P * r, P * (r + 1))
                    nc.tensor.matmul(
                        pgr, xa[:, sl], xa[:, P * r : D],
                        start=True, stop=False,
                    )
                    nc.tensor.matmul(
                        pgr[:, 0:P], idc[:, 0:P], idc[:, P : 2 * P],
                        start=False, stop=False, skip_group_check=True,
                    )
                    nc.tensor.matmul(
                        pgr, xp[:, sl], xp[:, D + P * r : 2 * D],
                        start=False, stop=True,
                    )

                def stats(r):
                    # row sums col r: diag-block colsum (PE) + earlier-block
                    # colsums (PE) + off-diag tail accum (DVE)
                    w = CH_W[r]
                    dc = dcs[r]
                    nc.tensor.matmul(
                        sm[:, r : r + 1], dc[:, 0:P], cbf[:],
                        start=True, stop=(r == 0),
                    )
                    if r < NCH - 1:
                        nc.vector.tensor_scalar(
                            dc[:, P:w], dc[:, P:w], 1.0, None, ALU.mult, ALU.add,
                            accum_out=acc4[:, r : r + 1],
                        )
                    for rp in range(r):
                        loc = P * (r - rp)
                        nc.tensor.matmul(
                            sm[:, r : r + 1],
                            dcs[rp][:, loc : loc + P],
                            cbf[:],
                            start=False, stop=(rp == r - 1),
                        )

                for r in (0, 1):
                    pgt = pg_pool.tile([P, D], f32, tag="pg")
                    pgr = pgt[:, 0 : CH_W[r]]
                    gram(r, pgr)
                    nc.scalar.activation(
                        dcs[r], pgr, AF.Sqrt, bias=bias_ap, scale=sc_ap
                    )
                    stats(r)
                # blocks 3+2 share one bank-sized PSUM tile and one sqrt
                # (contiguous in ob at 896:1280)
                pgb = pgb_pool.tile([P, 384], f32, tag="pgb")
                gram(3, pgb[:, 0:P])
                gram(2, pgb[:, P : P + 256])
                nc.scalar.activation(
                    ob[:, 896:OW], pgb[:], AF.Sqrt, bias=bias_ap, scale=sc_ap
                )
                stats(2)
                stats(3)
                # merge rowsum pieces into the output's trailing f32 cols
                rsv = ob[:, OW : OW + 8].bitcast(f32)
                nc.vector.tensor_tensor(
                    rsv[:, 0:3], acc4[:, 0:3], sm[:, 0:3], ALU.add
                )
                nc.vector.tensor_copy(rsv[:, 3:4], sm[:, 3:4])

            def store(prev, split=False):
                ps, pns, pob = prev
                if not split:
                    # dcov blocks (gated only by the sqrts) ship separately
                    # from the tiny rowsum cols (gated by the stats chain)
                    nc.sync.dma_start(
                        out_d[ps : ps + pns, :, 0:OW].rearrange("s p c -> p s c"),
                        pob[:].rearrange("p (s c) -> p s c", c=OWX)[:, :, 0:OW],
                    )
                    nc.sync.dma_start(
                        out_d[ps : ps + pns, :, OW:OWX].rearrange("s p c -> p s c"),
                        pob[:].rearrange("p (s c) -> p s c", c=OWX)[:, :, OW:OWX],
                    )
                    return
                # tail store: blocks 0,1 (ready right after their sqrts)
                # first, so only the short remainder trails the last compute
                nc.sync.dma_start(
                    out_d[ps : ps + pns, :, 0:896].rearrange("s p c -> p s c"),
                    pob[:].rearrange("p (s c) -> p s c", c=OWX)[:, :, 0:896],
                )
                nc.sync.dma_start(
                    out_d[ps : ps + pns, :, 896:OWX].rearrange("s p c -> p s c"),
                    pob[:].rearrange("p (s c) -> p s c", c=OWX)[:, :, 896:OWX],
                )

            # group sizes: small first/last groups shorten the pipeline
            # ramp (first sqrt waits on the first loads) and the drain
            # (the final stores); 4-sample groups amortize HWDGE otherwise.
            if n_samples < 12:
                sizes = [1] * n_samples
            else:
                body = n_samples - 8
                sizes = (
                    [1, 1, 2]
                    + [5] * (body // 5)
                    + ([body % 5] if body % 5 else [])
                    + [2, 1, 1]
                )

            prev = None
            s = 0
            for gi, ns in enumerate(sizes):
                if prev is not None:
                    store(prev)
                    prev = None
                xa2, xp2, ob2 = load_group(s, ns)
                if gi == 0:
                    # constants load after the first x loads: sample 0's
                    # gram starts as soon as xa/xp land (idc is only needed
                    # by its middle matmul, scb only by its sqrt)
                    nc.sync.dma_start(cst[:], cst_d[:])
                    nc.scalar.activation(et128[:], t128[:], AF.Exp)
                    nc.vector.tensor_scalar_mul(scb[:, 0:1], et128[:], -2.0)
                    nc.vector.tensor_scalar(
                        scb[:, 1:2], et128[:], 2.0 * KC, EPS, ALU.mult, ALU.add
                    )
                for k in range(ns):
                    stage1(k, xa2, xp2, ob2)
                prev = (s, ns, ob2)
                s += ns
            store(prev, split=True)
            assert s == n_samples, (sizes, s)

    # This walrus build accepts at most ONE sync wait per instruction.
    # Tile may attach several; hoist each extra wait onto its own no-op
    # placed just before the instruction (same engine, so ordering holds).
    if fixup:
        import concourse.mybir as mybir
        import bass_rust as _br

        for f in nc.m.functions:
            for blk in f.blocks:
                out_list = []
                changed = False
                for ins in blk.instructions:
                    si = getattr(ins, "sync_info", None)
                    if (
                        type(ins).__name__ != "InstNoOp"
                        and si is not None
                        and si.on_wait
                        and len(si.on_wait) > 1
                        and getattr(ins, "engine", None) is not None
                    ):
                        for j, w in enumerate(si.on_wait[:-1]):
                            nop = _br.InstNoOp(
                                name=f"I-w{j}-{ins.name}",
                                engine=ins.engine,
                                ins=[],
                                outs=[],
                            )
                            nop.sync_info = mybir.SyncInfo(on_wait=[w], on_update=[])
                            out_list.append(nop)
                        ins.sync_info = mybir.SyncInfo(
                            on_wait=[si.on_wait[-1]], on_update=list(si.on_update)
                        )
                        changed = True
                    out_list.append(ins)
                if changed:
                    blk.instructions = out_list
    return nc


def make_cst(t):
    """Packed per-run constants + the diagonal offset delta."""
    import ml_dtypes

    et = np.float32(np.exp(np.float32(np.asarray(t).reshape(-1)[0])))
    cval = np.float32(2.0 * GAMMA * et + EPS)
    delta = np.float32(np.sqrt(cval) - np.sqrt(np.float32(EPS)))
    idc = np.zeros((P, 2 * P), dtype=np.float32)
    idc[:, 0:P] = np.eye(P)
    idc[:, P : 2 * P] = -GAMMA * np.eye(P)
    cst = np.zeros((P, 520), dtype=np.uint8)
    cst[:, 0:512] = idc.astype(ml_dtypes.bfloat16).view(np.uint8)
    cst[:, 512:514] = np.ones((P, 1), dtype=np.float16).view(np.uint8)
    t128 = np.broadcast_to(np.asarray(t, dtype=np.float32).reshape(1, 1), (P, 1))
    cst[:, 516:520] = np.ascontiguousarray(t128).view(np.uint8)
    return cst, float(delta)


def prep_x(x):
    """Full x [B, M, D] f32 -> (xa bf16 [B,P,D], xp bf16 [B,KP,2D])."""
    import ml_dtypes

    xb16 = x.astype(ml_dtypes.bfloat16)
    xa = np.ascontiguousarray(xb16[:, 0:P, :])
    xbt = xb16[:, P:M, :]  # [B, 68, D]
    # u = (d - 196)/2 from the bf16-rounded x (consistent with device gram)
    d = np.square(xb16.astype(np.float32)).sum(axis=1)  # [B, D] f32
    u = ((d - KC) * 0.5).astype(ml_dtypes.bfloat16)
    xp = np.empty((x.shape[0], KP, 2 * D), dtype=ml_dtypes.bfloat16)
    xp[:, 0:MB, 0:D] = xbt
    xp[:, MB, 0:D] = u
    xp[:, MB + 1, 0:D] = ml_dtypes.bfloat16(-1.0)
    xp[:, 0:MB, D : 2 * D] = xbt
    xp[:, MB, D : 2 * D] = ml_dtypes.bfloat16(-1.0)
    xp[:, MB + 1, D : 2 * D] = u
    return xa, xp


def make_in_maps(x, t):
    cst, delta = make_cst(t)
    xa, xp = prep_x(np.ascontiguousarray(x, dtype=np.float32))
    in_maps = []
    for c in range(NCORES):
        sl = slice(c * S, (c + 1) * S)
        in_maps.append({"xa": xa[sl], "xp": xp[sl], "cst": cst})
    return in_maps, delta


# triu assembly indices (static)
_TRIU_ROWSTART = np.zeros(D + 1, dtype=np.int64)
for _i in range(D):
    _TRIU_ROWSTART[_i + 1] = _TRIU_ROWSTART[_i] + (D - _i)
TRIU_LEN = int(_TRIU_ROWSTART[D])  # 131328


def assemble(dev_out, delta):
    """[n, P, OWX] f16 device blocks+rowsums -> centered [n, 131328] triu."""
    n = dev_out.shape[0]
    dev_out = np.asarray(dev_out)
    rs = dev_out[:, :, OW : OW + 8].copy().view(np.float32)  # [n, P, 4]
    dc = dev_out[:, :, 0:OW].astype(np.float32)
    # rs[:, p, c] belongs to row index i = 128c + p
    c_full = rs.transpose(0, 2, 1).reshape(n, D) * (1.0 / D)  # rowmeans
    mu_true = (rs.sum(axis=(1, 2)) - D * delta) / (D * D)
    K = (mu_true + 2.0 * delta / D)[:, None]  # cent = dc - rm_i - rm_j + K
    out = np.empty((n, TRIU_LEN), dtype=np.float32)
    for r in range(NCH):
        for p in range(P):
            i = P * r + p
            st = _TRIU_ROWSTART[i]
            ln = D - i
            row = dc[:, p, OO[r] + p : OO[r] + p + ln]
            out[:, st : st + ln] = row - c_full[:, i : i + 1] - c_full[:, i:] + K
            out[:, st] -= delta  # gamma-shifted diagonal entry
    return out


_CACHE = {}


def kernel(**inputs):
    import concourse.bass_utils as bass_utils

    x = np.ascontiguousarray(inputs["x"], dtype=np.float32)
    t = np.asarray(inputs["t"], dtype=np.float32)
    assert x.shape == (B, M, D)

    if "nc" not in _CACHE:
        _CACHE["nc"] = build_nc(S)
    nc = _CACHE["nc"]
    in_maps, delta = make_in_maps(x, t)
    res = bass_utils.run_bass_kernel_spmd(nc, in_maps, core_ids=list(range(NCORES)))
    full = np.empty((B, TRIU_LEN), dtype=np.float32)
    for c in range(NCORES):
        full[c * S : (c + 1) * S] = assemble(res.results[c]["out"], delta)
    return full


# revision 49
# speedup vs baseline: 1.2145x; 1.0028x over previous
"""BDCovpool + Triuvec kernel for Trainium2 (8 NeuronCores, data-parallel).

Math (per sample b, x[b]: [M=196, D=512], t: scalar):
  gram[i,j] = sum_m x[m,i] x[m,j]           (D x D)
  d[i]      = gram[i,i]
  dpre      = d[i] + d[j] - 2 gram
  dcov      = sqrt(exp(t) * relu(dpre) + 1e-5)
  cent      = dcov - rowmean - colmean + totmean   (dcov symmetric -> row==col)
  out       = upper triangle of cent, row-major (131328 per sample)

Device strategy per core (32 samples), bf16/f16 datapath:
  x is shipped bf16.  The k-dim (m=196) splits 128 + 70 where the 70-row
  chunk carries two extra host-built rows: lhs [xb; u; -1], rhs
  [xb; -1; u], u = (d-196)/2 in bf16 (d precomputed from the bf16 x on
  the host).  Those rows contribute -u_i - u_j to the gram, so
      PSUM pg = gram - u_i - u_j - gamma*I_blockdiag
  and dcov = Sqrt(pg * (-2 e^t) + (392 e^t + eps)) with a constant bias;
  the gamma shift keeps the (cancelled, ~0) diagonal argument strictly
  positive.  Only the upper-triangle row blocks [128 x (512-128r)] are
  computed; each Sqrt writes f16 straight into the packed output tile.
  Row sums (for the centering statistics) combine three pieces:
   - diag-block column sums     (PE matmuls vs a ones column; the block
     is symmetric so colsum == rowsum)
   - earlier-block column sums  (PE, the mirrored lower-triangle part)
   - off-diag tail row sums     (DVE tensor_scalar accumulate, 4x mode)
  merged into 8 extra f16 (bitcast f32) columns of the output tile.
  4 consecutive samples share each input/output DMA (strided dram APs)
  to amortize the serialized HWDGE descriptor-generation cost.
Host: shard B across 8 cores, cast x to bf16 + build u rows, then during
un-sharding apply the (affine) double-centering from the shipped row
sums: cent = dcov - rm_i - rm_j + K, fix the gamma-shifted diagonal, and
gather the row-major upper triangle.
"""

import numpy as np

B, M, D = 256, 196, 512
NCORES = 8
S = B // NCORES  # samples per core
P = 128
NCH = D // P  # 4 row chunks
MB = M - P  # 68 rows in second k-chunk
KP = MB + 2  # 70 rows incl. u and -1
GAMMA = 256.0  # diagonal shift (bf16-exact)
EPS = 1e-5
KC = float(M)  # E[d] = M
CH_W = [D - P * r for r in range(NCH)]  # 512, 384, 256, 128
# packed block offsets; order 0,1,3,2 keeps each [128 x w] f32 PSUM block
# inside 2KB bank boundaries
OO = [0, 512, 1024, 896]
OW = 1280
OWX = OW + 8  # + 8 f16 cols holding the f32 [128,4] row sums (bitcast)


def build_nc(n_samples=S, fixup=True):
    import concourse.bass as bass
    import concourse.mybir as mybir
    import concourse.tile as tile

    f32 = mybir.dt.float32
    bf16 = mybir.dt.bfloat16
    f16 = mybir.dt.float16
    u8 = mybir.dt.uint8
    AF = mybir.ActivationFunctionType
    ALU = mybir.AluOpType

    nc = bass.Bass(
        "TRN2", target_bir_lowering=False, debug=False, enable_asserts=False
    )

    xa_d = nc.dram_tensor("xa", [n_samples, P, D], bf16, kind="ExternalInput").ap()
    xp_d = nc.dram_tensor("xp", [n_samples, KP, 2 * D], bf16, kind="ExternalInput").ap()
    # packed constants: idc = [I | -gamma*I] bf16 (512B), ones f16 (2B),
    # t f32 (4B at byte 516)
    cst_d = nc.dram_tensor("cst", [P, 520], u8, kind="ExternalInput").ap()
    out_d = nc.dram_tensor("out", [n_samples, P, OWX], f16, kind="ExternalOutput").ap()

    with tile.TileContext(nc) as tc:
        with (
            tc.tile_pool(name="const", bufs=1) as cpool,
            tc.tile_pool(name="xa", bufs=6) as xa_pool,
            tc.tile_pool(name="xp", bufs=6) as xp_pool,
            tc.tile_pool(name="sm", bufs=4) as sm_pool,
            tc.tile_pool(name="uo", bufs=10) as uo_pool,
            tc.tile_pool(name="pg", bufs=3, space="PSUM") as pg_pool,
            tc.tile_pool(name="pgb", bufs=2, space="PSUM") as pgb_pool,
            tc.tile_pool(name="psm", bufs=3, space="PSUM") as psm_pool,
        ):
            cst = cpool.tile([P, 520], u8, tag="cst")
            idc = cst[:, 0:512].bitcast(bf16)
            cbf = cst[:, 512:514].bitcast(f16)
            t128 = cst[:, 516:520].bitcast(f32)
            et128 = cpool.tile([P, 1], f32, tag="et128")
            scb = cpool.tile([P, 2], f32, tag="scb")
            # scb col0 = -2 e^t (sqrt scale); col1 = 392 e^t + 1e-5 (bias);
            # filled after the first loads are issued (see main loop)
            sc_ap = scb[:, 0:1]
            bias_ap = scb[:, 1:2]

            def load_group(s, ns):
                """One DMA per dram tensor for `ns` consecutive samples."""
                xa = xa_pool.tile([P, ns * D], bf16, tag="xa")
                nc.sync.dma_start(
                    xa[:].rearrange("p (s c) -> p s c", s=ns),
                    xa_d[s : s + ns].rearrange("s p c -> p s c"),
                )
                xp = xp_pool.tile([KP, ns * 2 * D], bf16, tag="xp")
                nc.sync.dma_start(
                    xp[:].rearrange("p (s c) -> p s c", s=ns),
                    xp_d[s : s + ns].rearrange("s p c -> p s c"),
                )
                ob = uo_pool.tile([P, ns * OWX], f16, tag="ob")
                return xa, xp, ob

            def stage1(k, xa2, xp2, ob2):
                xa = xa2[:, k * D : (k + 1) * D]
                xp = xp2[:, k * 2 * D : (k + 1) * 2 * D]
                ob = ob2[:, k * OWX : (k + 1) * OWX]
                acc4 = sm_pool.tile([P, 4], f32, tag="acc4")
                smt = psm_pool.tile([P, 4], f32, tag="smt")
                sm = smt[:, 0:4]
                dcs = [ob[:, OO[r] : OO[r] + CH_W[r]] for r in range(NCH)]

                def gram(r, pgr):
                    sl = slice(P * r, P * (r + 1))
                    nc.tensor.matmul(
                        pgr, xa[:, sl], xa[:, P * r : D],
                        start=True, stop=False,
                    )
                    nc.tensor.matmul(
                        pgr[:, 0:P], idc[:, 0:P], idc[:, P : 2 * P],
                        start=False, stop=False, skip_group_check=True,
                    )
                    nc.tensor.matmul(
                        pgr, xp[:, sl], xp[:, D + P * r : 2 * D],
                        start=False, stop=True,
                    )

                def stats(r):
                    # row sums col r: diag-block colsum (PE) + earlier-block
                    # colsums (PE) + off-diag tail accum (DVE)
                    w = CH_W[r]
                    dc = dcs[r]
                    nc.tensor.matmul(
                        sm[:, r : r + 1], dc[:, 0:P], cbf[:],
                        start=True, stop=(r == 0),
                    )
                    if r < NCH - 1:
                        nc.vector.tensor_scalar(
                            dc[:, P:w], dc[:, P:w], 1.0, None, ALU.mult, ALU.add,
                            accum_out=acc4[:, r : r + 1],
                        )
                    for rp in range(r):
                        loc = P * (r - rp)
                        nc.tensor.matmul(
                            sm[:, r : r + 1],
                            dcs[rp][:, loc : loc + P],
                            cbf[:],
                            start=False, stop=(rp == r - 1),
                        )

                for r in (0, 1):
                    pgt = pg_pool.tile([P, D], f32, tag="pg")
                    pgr = pgt[:, 0 : CH_W[r]]
                    gram(r, pgr)
                    nc.scalar.activation(
                        dcs[r], pgr, AF.Sqrt, bias=bias_ap, scale=sc_ap
                    )
                    stats(r)
                # blocks 3+2 share one bank-sized PSUM tile and one sqrt
                # (contiguous in ob at 896:1280)
                pgb = pgb_pool.tile([P, 384], f32, tag="pgb")
                gram(3, pgb[:, 0:P])
                gram(2, pgb[:, P : P + 256])
                nc.scalar.activation(
                    ob[:, 896:OW], pgb[:], AF.Sqrt, bias=bias_ap, scale=sc_ap
                )
                stats(2)
                stats(3)
                # merge rowsum pieces into the output's trailing f32 cols
                rsv = ob[:, OW : OW + 8].bitcast(f32)
                nc.vector.tensor_tensor(
                    rsv[:, 0:3], acc4[:, 0:3], sm[:, 0:3], ALU.add
                )
                nc.vector.tensor_copy(rsv[:, 3:4], sm[:, 3:4])

            def store(prev, split=False):
                ps, pns, pob = prev
                if not split:
                    # dcov blocks (gated only by the sqrts) ship separately
                    # from the tiny rowsum cols (gated by the stats chain)
                    nc.sync.dma_start(
                        out_d[ps : ps + pns, :, 0:OW].rearrange("s p c -> p s c"),
                        pob[:].rearrange("p (s c) -> p s c", c=OWX)[:, :, 0:OW],
                    )
                    nc.sync.dma_start(
                        out_d[ps : ps + pns, :, OW:OWX].rearrange("s p c -> p s c"),
                        pob[:].rearrange("p (s c) -> p s c", c=OWX)[:, :, OW:OWX],
                    )
                    return
                # tail store: blocks 0,1 (ready right after their sqrts)
                # first, so only the short remainder trails the last compute
                nc.sync.dma_start(
                    out_d[ps : ps + pns, :, 0:896].rearrange("s p c -> p s c"),
                    pob[:].rearrange("p (s c) -> p s c", c=OWX)[:, :, 0:896],
                )
                nc.scalar.dma_start(
                    out_d[ps : ps + pns, :, 896:OWX].rearrange("s p c -> p s c"),
                    pob[:].rearrange("p (s c) -> p s c", c=OWX)[:, :, 896:OWX],
                )

            # group sizes: small first/last groups shorten the pipeline
            # ramp (first sqrt waits on the first loads) and the drain
            # (the final stores); 4-sample groups amortize HWDGE otherwise.
            if n_samples < 12:
                sizes = [1] * n_samples
            else:
                body = n_samples - 8
                sizes = (
                    [1, 1, 2]
                    + [5] * (body // 5)
                    + ([body % 5] if body % 5 else [])
                    + [2, 1, 1]
                )

            prev = None
            s = 0
            for gi, ns in enumerate(sizes):
                if prev is not None:
                    store(prev)
                    prev = None
                xa2, xp2, ob2 = load_group(s, ns)
                if gi == 0:
                    # constants load after the first x loads: sample 0's
                    # gram starts as soon as xa/xp land (idc is only needed
                    # by its middle matmul, scb only by its sqrt)
                    nc.sync.dma_start(cst[:], cst_d[:])
                    nc.scalar.activation(et128[:], t128[:], AF.Exp)
                    nc.vector.tensor_scalar_mul(scb[:, 0:1], et128[:], -2.0)
                    nc.vector.tensor_scalar(
                        scb[:, 1:2], et128[:], 2.0 * KC, EPS, ALU.mult, ALU.add
                    )
                for k in range(ns):
                    stage1(k, xa2, xp2, ob2)
                prev = (s, ns, ob2)
                s += ns
            store(prev, split=True)
            assert s == n_samples, (sizes, s)

    # This walrus build accepts at most ONE sync wait per instruction.
    # Tile may attach several; hoist each extra wait onto its own no-op
    # placed just before the instruction (same engine, so ordering holds).
    if fixup:
        import concourse.mybir as mybir
        import bass_rust as _br

        for f in nc.m.functions:
            for blk in f.blocks:
                out_list = []
                changed = False
                for ins in blk.instructions:
                    si = getattr(ins, "sync_info", None)
                    if (
                        type(ins).__name__ != "InstNoOp"
                        and si is not None
                        and si.on_wait
                        and len(si.on_wait) > 1
                        and getattr(ins, "engine", None) is not None
                    ):
                        for j, w in enumerate(si.on_wait[:-1]):
                            nop = _br.InstNoOp(
                                name=f"I-w{j}-{ins.name}",
                                engine=ins.engine,
                                ins=[],
                                outs=[],
                            )
                            nop.sync_info = mybir.SyncInfo(on_wait=[w], on_update=[])
                            out_list.append(nop)
                        ins.sync_info = mybir.SyncInfo(
                            on_wait=[si.on_wait[-1]], on_update=list(si.on_update)
                        )
                        changed = True
                    out_list.append(ins)
                if changed:
                    blk.instructions = out_list
    return nc


def make_cst(t):
    """Packed per-run constants + the diagonal offset delta."""
    import ml_dtypes

    et = np.float32(np.exp(np.float32(np.asarray(t).reshape(-1)[0])))
    cval = np.float32(2.0 * GAMMA * et + EPS)
    delta = np.float32(np.sqrt(cval) - np.sqrt(np.float32(EPS)))
    idc = np.zeros((P, 2 * P), dtype=np.float32)
    idc[:, 0:P] = np.eye(P)
    idc[:, P : 2 * P] = -GAMMA * np.eye(P)
    cst = np.zeros((P, 520), dtype=np.uint8)
    cst[:, 0:512] = idc.astype(ml_dtypes.bfloat16).view(np.uint8)
    cst[:, 512:514] = np.ones((P, 1), dtype=np.float16).view(np.uint8)
    t128 = np.broadcast_to(np.asarray(t, dtype=np.float32).reshape(1, 1), (P, 1))
    cst[:, 516:520] = np.ascontiguousarray(t128).view(np.uint8)
    return cst, float(delta)


def prep_x(x):
    """Full x [B, M, D] f32 -> (xa bf16 [B,P,D], xp bf16 [B,KP,2D])."""
    import ml_dtypes

    xb16 = x.astype(ml_dtypes.bfloat16)
    xa = np.ascontiguousarray(xb16[:, 0:P, :])
    xbt = xb16[:, P:M, :]  # [B, 68, D]
    # u = (d - 196)/2 from the bf16-rounded x (consistent with device gram)
    d = np.square(xb16.astype(np.float32)).sum(axis=1)  # [B, D] f32
    u = ((d - KC) * 0.5).astype(ml_dtypes.bfloat16)
    xp = np.empty((x.shape[0], KP, 2 * D), dtype=ml_dtypes.bfloat16)
    xp[:, 0:MB, 0:D] = xbt
    xp[:, MB, 0:D] = u
    xp[:, MB + 1, 0:D] = ml_dtypes.bfloat16(-1.0)
    xp[:, 0:MB, D : 2 * D] = xbt
    xp[:, MB, D : 2 * D] = ml_dtypes.bfloat16(-1.0)
    xp[:, MB + 1, D : 2 * D] = u
    return xa, xp


def make_in_maps(x, t):
    cst, delta = make_cst(t)
    xa, xp = prep_x(np.ascontiguousarray(x, dtype=np.float32))
    in_maps = []
    for c in range(NCORES):
        sl = slice(c * S, (c + 1) * S)
        in_maps.append({"xa": xa[sl], "xp": xp[sl], "cst": cst})
    return in_maps, delta


# triu assembly indices (static)
_TRIU_ROWSTART = np.zeros(D + 1, dtype=np.int64)
for _i in range(D):
    _TRIU_ROWSTART[_i + 1] = _TRIU_ROWSTART[_i] + (D - _i)
TRIU_LEN = int(_TRIU_ROWSTART[D])  # 131328


def assemble(dev_out, delta):
    """[n, P, OWX] f16 device blocks+rowsums -> centered [n, 131328] triu."""
    n = dev_out.shape[0]
    dev_out = np.asarray(dev_out)
    rs = dev_out[:, :, OW : OW + 8].copy().view(np.float32)  # [n, P, 4]
    dc = dev_out[:, :, 0:OW].astype(np.float32)
    # rs[:, p, c] belongs to row index i = 128c + p
    c_full = rs.transpose(0, 2, 1).reshape(n, D) * (1.0 / D)  # rowmeans
    mu_true = (rs.sum(axis=(1, 2)) - D * delta) / (D * D)
    K = (mu_true + 2.0 * delta / D)[:, None]  # cent = dc - rm_i - rm_j + K
    out = np.empty((n, TRIU_LEN), dtype=np.float32)
    for r in range(NCH):
        for p in range(P):
            i = P * r + p
            st = _TRIU_ROWSTART[i]
            ln = D - i
            row = dc[:, p, OO[r] + p : OO[r] + p + ln]
            out[:, st : st + ln] = row - c_full[:, i : i + 1] - c_full[:, i:] + K
            out[:, st] -= delta  # gamma-shifted diagonal entry
    return out


_CACHE = {}


def kernel(**inputs):
    import concourse.bass_utils as bass_utils

    x = np.ascontiguousarray(inputs["x"], dtype=np.float32)
    t = np.asarray(inputs["t"], dtype=np.float32)
    assert x.shape == (B, M, D)

    if "nc" not in _CACHE:
        _CACHE["nc"] = build_nc(S)
    nc = _CACHE["nc"]
    in_maps, delta = make_in_maps(x, t)
    res = bass_utils.run_bass_kernel_spmd(nc, in_maps, core_ids=list(range(NCORES)))
    full = np.empty((B, TRIU_LEN), dtype=np.float32)
    for c in range(NCORES):
        full[c * S : (c + 1) * S] = assemble(res.results[c]["out"], delta)
    return full


# revision 50
# speedup vs baseline: 1.2169x; 1.0020x over previous
"""BDCovpool + Triuvec kernel for Trainium2 (8 NeuronCores, data-parallel).

Math (per sample b, x[b]: [M=196, D=512], t: scalar):
  gram[i,j] = sum_m x[m,i] x[m,j]           (D x D)
  d[i]      = gram[i,i]
  dpre      = d[i] + d[j] - 2 gram
  dcov      = sqrt(exp(t) * relu(dpre) + 1e-5)
  cent      = dcov - rowmean - colmean + totmean   (dcov symmetric -> row==col)
  out       = upper triangle of cent, row-major (131328 per sample)

Device strategy per core (32 samples), bf16/f16 datapath:
  x is shipped bf16.  The k-dim (m=196) splits 128 + 70 where the 70-row
  chunk carries two extra host-built rows: lhs [xb; u; -1], rhs
  [xb; -1; u], u = (d-196)/2 in bf16 (d precomputed from the bf16 x on
  the host).  Those rows contribute -u_i - u_j to the gram, so
      PSUM pg = gram - u_i - u_j - gamma*I_blockdiag
  and dcov = Sqrt(pg * (-2 e^t) + (392 e^t + eps)) with a constant bias;
  the gamma shift keeps the (cancelled, ~0) diagonal argument strictly
  positive.  Only the upper-triangle row blocks [128 x (512-128r)] are
  computed; each Sqrt writes f16 straight into the packed output tile.
  Row sums (for the centering statistics) combine three pieces:
   - diag-block column sums     (PE matmuls vs a ones column; the block
     is symmetric so colsum == rowsum)
   - earlier-block column sums  (PE, the mirrored lower-triangle part)
   - off-diag tail row sums     (DVE tensor_scalar accumulate, 4x mode)
  merged into 8 extra f16 (bitcast f32) columns of the output tile.
  4 consecutive samples share each input/output DMA (strided dram APs)
  to amortize the serialized HWDGE descriptor-generation cost.
Host: shard B across 8 cores, cast x to bf16 + build u rows, then during
un-sharding apply the (affine) double-centering from the shipped row
sums: cent = dcov - rm_i - rm_j + K, fix the gamma-shifted diagonal, and
gather the row-major upper triangle.
"""

import numpy as np

B, M, D = 256, 196, 512
NCORES = 8
S = B // NCORES  # samples per core
P = 128
NCH = D // P  # 4 row chunks
MB = M - P  # 68 rows in second k-chunk
KP = MB + 2  # 70 rows incl. u and -1
GAMMA = 256.0  # diagonal shift (bf16-exact)
EPS = 1e-5
KC = float(M)  # E[d] = M
CH_W = [D - P * r for r in range(NCH)]  # 512, 384, 256, 128
# packed block offsets; order 0,1,3,2 keeps each [128 x w] f32 PSUM block
# inside 2KB bank boundaries
OO = [0, 512, 1024, 896]
OW = 1280
OWX = OW + 8  # + 8 f16 cols holding the f32 [128,4] row sums (bitcast)


def build_nc(n_samples=S, fixup=True):
    import concourse.bass as bass
    import concourse.mybir as mybir
    import concourse.tile as tile

    f32 = mybir.dt.float32
    bf16 = mybir.dt.bfloat16
    f16 = mybir.dt.float16
    u8 = mybir.dt.uint8
    AF = mybir.ActivationFunctionType
    ALU = mybir.AluOpType

    nc = bass.Bass(
        "TRN2", target_bir_lowering=False, debug=False, enable_asserts=False
    )

    xa_d = nc.dram_tensor("xa", [n_samples, P, D], bf16, kind="ExternalInput").ap()
    xp_d = nc.dram_tensor("xp", [n_samples, KP, 2 * D], bf16, kind="ExternalInput").ap()
    # packed constants: idc = [I | -gamma*I] bf16 (512B), ones f16 (2B),
    # t f32 (4B at byte 516)
    cst_d = nc.dram_tensor("cst", [P, 520], u8, kind="ExternalInput").ap()
    out_d = nc.dram_tensor("out", [n_samples, P, OWX], f16, kind="ExternalOutput").ap()

    with tile.TileContext(nc) as tc:
        with (
            tc.tile_pool(name="const", bufs=1) as cpool,
            tc.tile_pool(name="xa", bufs=6) as xa_pool,
            tc.tile_pool(name="xp", bufs=6) as xp_pool,
            tc.tile_pool(name="sm", bufs=4) as sm_pool,
            tc.tile_pool(name="uo", bufs=10) as uo_pool,
            tc.tile_pool(name="pg", bufs=3, space="PSUM") as pg_pool,
            tc.tile_pool(name="pgb", bufs=2, space="PSUM") as pgb_pool,
            tc.tile_pool(name="psm", bufs=3, space="PSUM") as psm_pool,
        ):
            cst = cpool.tile([P, 520], u8, tag="cst")
            idc = cst[:, 0:512].bitcast(bf16)
            cbf = cst[:, 512:514].bitcast(f16)
            t128 = cst[:, 516:520].bitcast(f32)
            et128 = cpool.tile([P, 1], f32, tag="et128")
            scb = cpool.tile([P, 2], f32, tag="scb")
            # scb col0 = -2 e^t (sqrt scale); col1 = 392 e^t + 1e-5 (bias);
            # filled after the first loads are issued (see main loop)
            sc_ap = scb[:, 0:1]
            bias_ap = scb[:, 1:2]

            def load_group(s, ns):
                """One DMA per dram tensor for `ns` consecutive samples."""
                xa = xa_pool.tile([P, ns * D], bf16, tag="xa")
                nc.sync.dma_start(
                    xa[:].rearrange("p (s c) -> p s c", s=ns),
                    xa_d[s : s + ns].rearrange("s p c -> p s c"),
                )
                xp = xp_pool.tile([KP, ns * 2 * D], bf16, tag="xp")
                nc.sync.dma_start(
                    xp[:].rearrange("p (s c) -> p s c", s=ns),
                    xp_d[s : s + ns].rearrange("s p c -> p s c"),
                )
                ob = uo_pool.tile([P, ns * OWX], f16, tag="ob")
                return xa, xp, ob

            def stage1(k, xa2, xp2, ob2):
                xa = xa2[:, k * D : (k + 1) * D]
                xp = xp2[:, k * 2 * D : (k + 1) * 2 * D]
                ob = ob2[:, k * OWX : (k + 1) * OWX]
                acc4 = sm_pool.tile([P, 4], f32, tag="acc4")
                smt = psm_pool.tile([P, 4], f32, tag="smt")
                sm = smt[:, 0:4]
                dcs = [ob[:, OO[r] : OO[r] + CH_W[r]] for r in range(NCH)]

                def gram(r, pgr):
                    sl = slice(P * r, P * (r + 1))
                    nc.tensor.matmul(
                        pgr, xa[:, sl], xa[:, P * r : D],
                        start=True, stop=False,
                    )
                    nc.tensor.matmul(
                        pgr[:, 0:P], idc[:, 0:P], idc[:, P : 2 * P],
                        start=False, stop=False, skip_group_check=True,
                    )
                    nc.tensor.matmul(
                        pgr, xp[:, sl], xp[:, D + P * r : 2 * D],
                        start=False, stop=True,
                    )

                def stats(r):
                    # row sums col r: diag-block colsum (PE) + earlier-block
                    # colsums (PE) + off-diag tail accum (DVE)
                    w = CH_W[r]
                    dc = dcs[r]
                    nc.tensor.matmul(
                        sm[:, r : r + 1], dc[:, 0:P], cbf[:],
                        start=True, stop=(r == 0),
                    )
                    if r < NCH - 1:
                        nc.vector.tensor_scalar(
                            dc[:, P:w], dc[:, P:w], 1.0, None, ALU.mult, ALU.add,
                            accum_out=acc4[:, r : r + 1],
                        )
                    for rp in range(r):
                        loc = P * (r - rp)
                        nc.tensor.matmul(
                            sm[:, r : r + 1],
                            dcs[rp][:, loc : loc + P],
                            cbf[:],
                            start=False, stop=(rp == r - 1),
                        )

                for r in (0, 1):
                    pgt = pg_pool.tile([P, D], f32, tag="pg")
                    pgr = pgt[:, 0 : CH_W[r]]
                    gram(r, pgr)
                    nc.scalar.activation(
                        dcs[r], pgr, AF.Sqrt, bias=bias_ap, scale=sc_ap
                    )
                    stats(r)
                # blocks 3+2 share one bank-sized PSUM tile and one sqrt
                # (contiguous in ob at 896:1280)
                pgb = pgb_pool.tile([P, 384], f32, tag="pgb")
                gram(3, pgb[:, 0:P])
                gram(2, pgb[:, P : P + 256])
                nc.scalar.activation(
                    ob[:, 896:OW], pgb[:], AF.Sqrt, bias=bias_ap, scale=sc_ap
                )
                stats(2)
                stats(3)
                # merge rowsum pieces into the output's trailing f32 cols
                rsv = ob[:, OW : OW + 8].bitcast(f32)
                nc.vector.tensor_tensor(
                    rsv[:, 0:3], acc4[:, 0:3], sm[:, 0:3], ALU.add
                )
                nc.vector.tensor_copy(rsv[:, 3:4], sm[:, 3:4])

            def store(prev, split=False):
                ps, pns, pob = prev
                if not split:
                    # dcov blocks (gated only by the sqrts) ship separately
                    # from the tiny rowsum cols (gated by the stats chain)
                    nc.sync.dma_start(
                        out_d[ps : ps + pns, :, 0:OW].rearrange("s p c -> p s c"),
                        pob[:].rearrange("p (s c) -> p s c", c=OWX)[:, :, 0:OW],
                    )
                    nc.sync.dma_start(
                        out_d[ps : ps + pns, :, OW:OWX].rearrange("s p c -> p s c"),
                        pob[:].rearrange("p (s c) -> p s c", c=OWX)[:, :, OW:OWX],
                    )
                    return
                # tail store: blocks 0,1 (ready right after their sqrts)
                # first, so only the short remainder trails the last compute
                nc.sync.dma_start(
                    out_d[ps : ps + pns, :, 0:896].rearrange("s p c -> p s c"),
                    pob[:].rearrange("p (s c) -> p s c", c=OWX)[:, :, 0:896],
                )
                nc.scalar.dma_start(
                    out_d[ps : ps + pns, :, 896:OWX].rearrange("s p c -> p s c"),
                    pob[:].rearrange("p (s c) -> p s c", c=OWX)[:, :, 896:OWX],
                )

            # group sizes: small first/last groups shorten the pipeline
            # ramp (first sqrt waits on the first loads) and the drain
            # (the final stores); 4-sample groups amortize HWDGE otherwise.
            if n_samples < 12:
                sizes = [1] * n_samples
            else:
                body = n_samples - 8
                sizes = (
                    [1, 1, 2]
                    + [5] * (body // 5)
                    + ([body % 5] if body % 5 else [])
                    + [1, 1, 1, 1]
                )

            prev = None
            s = 0
            for gi, ns in enumerate(sizes):
                if prev is not None:
                    store(prev)
                    prev = None
                xa2, xp2, ob2 = load_group(s, ns)
                if gi == 0:
                    # constants load after the first x loads: sample 0's
                    # gram starts as soon as xa/xp land (idc is only needed
                    # by its middle matmul, scb only by its sqrt)
                    nc.sync.dma_start(cst[:], cst_d[:])
                    nc.scalar.activation(et128[:], t128[:], AF.Exp)
                    nc.vector.tensor_scalar_mul(scb[:, 0:1], et128[:], -2.0)
                    nc.vector.tensor_scalar(
                        scb[:, 1:2], et128[:], 2.0 * KC, EPS, ALU.mult, ALU.add
                    )
                for k in range(ns):
                    stage1(k, xa2, xp2, ob2)
                prev = (s, ns, ob2)
                s += ns
            store(prev, split=True)
            assert s == n_samples, (sizes, s)

    # This walrus build accepts at most ONE sync wait per instruction.
    # Tile may attach several; hoist each extra wait onto its own no-op
    # placed just before the instruction (same engine, so ordering holds).
    if fixup:
        import concourse.mybir as mybir
        import bass_rust as _br

        for f in nc.m.functions:
            for blk in f.blocks:
                out_list = []
                changed = False
                for ins in blk.instructions:
                    si = getattr(ins, "sync_info", None)
                    if (
                        type(ins).__name__ != "InstNoOp"
                        and si is not None
                        and si.on_wait
                        and len(si.on_wait) > 1
                        and getattr(ins, "engine", None) is not None
                    ):
                        for j, w in enumerate(si.on_wait[:-1]):
                            nop = _br.InstNoOp(
                                name=f"I-w{j}-{ins.name}",
                                engine=ins.engine,
                                ins=[],
                                outs=[],
                            )
                            nop.sync_info = mybir.SyncInfo(on_wait=[w], on_update=[])
                            out_list.append(nop)
                        ins.sync_info = mybir.SyncInfo(
                            on_wait=[si.on_wait[-1]], on_update=list(si.on_update)
                        )
                        changed = True
                    out_list.append(ins)
                if changed:
                    blk.instructions = out_list
    return nc


def make_cst(t):
    """Packed per-run constants + the diagonal offset delta."""
    import ml_dtypes

    et = np.float32(np.exp(np.float32(np.asarray(t).reshape(-1)[0])))
    cval = np.float32(2.0 * GAMMA * et + EPS)
    delta = np.float32(np.sqrt(cval) - np.sqrt(np.float32(EPS)))
    idc = np.zeros((P, 2 * P), dtype=np.float32)
    idc[:, 0:P] = np.eye(P)
    idc[:, P : 2 * P] = -GAMMA * np.eye(P)
    cst = np.zeros((P, 520), dtype=np.uint8)
    cst[:, 0:512] = idc.astype(ml_dtypes.bfloat16).view(np.uint8)
    cst[:, 512:514] = np.ones((P, 1), dtype=np.float16).view(np.uint8)
    t128 = np.broadcast_to(np.asarray(t, dtype=np.float32).reshape(1, 1), (P, 1))
    cst[:, 516:520] = np.ascontiguousarray(t128).view(np.uint8)
    return cst, float(delta)


def prep_x(x):
    """Full x [B, M, D] f32 -> (xa bf16 [B,P,D], xp bf16 [B,KP,2D])."""
    import ml_dtypes

    xb16 = x.astype(ml_dtypes.bfloat16)
    xa = np.ascontiguousarray(xb16[:, 0:P, :])
    xbt = xb16[:, P:M, :]  # [B, 68, D]
    # u = (d - 196)/2 from the bf16-rounded x (consistent with device gram)
    d = np.square(xb16.astype(np.float32)).sum(axis=1)  # [B, D] f32
    u = ((d - KC) * 0.5).astype(ml_dtypes.bfloat16)
    xp = np.empty((x.shape[0], KP, 2 * D), dtype=ml_dtypes.bfloat16)
    xp[:, 0:MB, 0:D] = xbt
    xp[:, MB, 0:D] = u
    xp[:, MB + 1, 0:D] = ml_dtypes.bfloat16(-1.0)
    xp[:, 0:MB, D : 2 * D] = xbt
    xp[:, MB, D : 2 * D] = ml_dtypes.bfloat16(-1.0)
    xp[:, MB + 1, D : 2 * D] = u
    return xa, xp


def make_in_maps(x, t):
    cst, delta = make_cst(t)
    xa, xp = prep_x(np.ascontiguousarray(x, dtype=np.float32))
    in_maps = []
    for c in range(NCORES):
        sl = slice(c * S, (c + 1) * S)
        in_maps.append({"xa": xa[sl], "xp": xp[sl], "cst": cst})
    return in_maps, delta


# triu assembly indices (static)
_TRIU_ROWSTART = np.zeros(D + 1, dtype=np.int64)
for _i in range(D):
    _TRIU_ROWSTART[_i + 1] = _TRIU_ROWSTART[_i] + (D - _i)
TRIU_LEN = int(_TRIU_ROWSTART[D])  # 131328


def assemble(dev_out, delta):
    """[n, P, OWX] f16 device blocks+rowsums -> centered [n, 131328] triu."""
    n = dev_out.shape[0]
    dev_out = np.asarray(dev_out)
    rs = dev_out[:, :, OW : OW + 8].copy().view(np.float32)  # [n, P, 4]
    dc = dev_out[:, :, 0:OW].astype(np.float32)
    # rs[:, p, c] belongs to row index i = 128c + p
    c_full = rs.transpose(0, 2, 1).reshape(n, D) * (1.0 / D)  # rowmeans
    mu_true = (rs.sum(axis=(1, 2)) - D * delta) / (D * D)
    K = (mu_true + 2.0 * delta / D)[:, None]  # cent = dc - rm_i - rm_j + K
    out = np.empty((n, TRIU_LEN), dtype=np.float32)
    for r in range(NCH):
        for p in range(P):
            i = P * r + p
            st = _TRIU_ROWSTART[i]
            ln = D - i
            row = dc[:, p, OO[r] + p : OO[r] + p + ln]
            out[:, st : st + ln] = row - c_full[:, i : i + 1] - c_full[:, i:] + K
            out[:, st] -= delta  # gamma-shifted diagonal entry
    return out


_CACHE = {}


def kernel(**inputs):
    import concourse.bass_utils as bass_utils

    x = np.ascontiguousarray(inputs["x"], dtype=np.float32)
    t = np.asarray(inputs["t"], dtype=np.float32)
    assert x.shape == (B, M, D)

    if "nc" not in _CACHE:
        _CACHE["nc"] = build_nc(S)
    nc = _CACHE["nc"]
    in_maps, delta = make_in_maps(x, t)
    res = bass_utils.run_bass_kernel_spmd(nc, in_maps, core_ids=list(range(NCORES)))
    full = np.empty((B, TRIU_LEN), dtype=np.float32)
    for c in range(NCORES):
        full[c * S : (c + 1) * S] = assemble(res.results[c]["out"], delta)
    return full
